# revision 2
# baseline (speedup 1.0000x reference)
"""Causal self-attention Trainium2 kernel v3 (8 NeuronCores, SPMD).

Problem (hardcoded): x [4, 2048, 2048] f32, W_qkv [6144, 2048], W_out [2048, 2048],
16 heads x 128 dim, causal softmax attention + output projection.

Sharding: core c = 2*b + g handles batch b (4) and head-group g (2 groups of 8
heads).  Host sums the two W_out-partial outputs per batch element.

v3 design:
- fp16 everywhere (PSUM f32): same PE rate as f32r, 2x DVE rate, ~0.03%
  quantization error, halves SBUF/DMA footprints
- no DRAM roundtrip: V (all heads) and per-head Q^T/K^T stay in SBUF
- V is computed first, then per head: QK projection immediately followed by
  that head's attention, so attention's Act/DVE load hides under the
  projection's PE time
- softmax normalization fully off-PE: DVE accumulates exp tiles (fp16, 2x),
  Pool partition_all_reduce forms the denominators, DVE reciprocal+multiply
- attention k-chunks processed in PAIRS sharing one [128,1024] PSUM tile so
  one Act exp instruction covers two chunks (Act per-instruction overhead
  ~190ns would otherwise starve the pipeline); causal-diagonal chunks pack
  (512+384) and (256+128) wide pairs with packed masks
"""

import math

import numpy as np

B = 4
T = 2048
C = 2048
H = 16          # total heads
HG = 8          # heads per core (tensor-parallel group)
D = 128         # head dim
P = 128         # partitions
NCS = C // P    # 16 contraction subtiles
NTC = T // P    # 16 T chunks of 128
NTB = T // 512  # 4 T blocks of 512
SCALE = 1.0 / math.sqrt(D)

_CACHED = None


def _build(phases="abc", repeat=1, vmode=None):
    import concourse.mybir as mybir
    from concourse import bacc
    from concourse import bass_isa
    from concourse.tile import TileContext

    f32 = mybir.dt.float32
    fp16 = mybir.dt.float16
    EXP = mybir.ActivationFunctionType.Exp
    MULT = mybir.AluOpType.mult
    ADD = mybir.AluOpType.add
    RADD = bass_isa.ReduceOp.add

    nc = bacc.Bacc("TRN2", target_bir_lowering=False)

    xt_d = nc.dram_tensor("xt", [NCS, P, T], fp16, kind="ExternalInput")
    wq_d = nc.dram_tensor("wq", [HG, P, NCS, D], fp16, kind="ExternalInput")
    wk_d = nc.dram_tensor("wk", [HG, P, NCS, D], fp16, kind="ExternalInput")
    wv_d = nc.dram_tensor("wv", [P, NCS, HG * D], fp16, kind="ExternalInput")
    wo_d = nc.dram_tensor("wo", [HG * D, C], fp16, kind="ExternalInput")
    maska_d = nc.dram_tensor("maska", [P, 896], fp16, kind="ExternalInput")
    maskb_d = nc.dram_tensor("maskb", [P, 384], fp16, kind="ExternalInput")
    out_d = nc.dram_tensor("out", [T, C], f32, kind="ExternalOutput")

    do_b = "b" in phases
    do_c = "c" in phases

    with TileContext(nc) as tc:
        with tc.tile_pool(name="persist", bufs=1) as persist:
            maska_t = persist.tile([P, 896], fp16, tag="maska")
            nc.sync.dma_start(maska_t, maska_d[:])
            maskb_t = persist.tile([P, 384], fp16, tag="maskb")
            nc.sync.dma_start(maskb_t, maskb_d[:])

            for _rep in range(repeat):
                with tc.tile_pool(name="vsb", bufs=1) as vsbp, \
                     tc.tile_pool(name="ot", bufs=1) as otp, \
                     tc.tile_pool(name="xt", bufs=1) as xtp, \
                     tc.tile_pool(name="astage", bufs=4) as astage:
                    v_sb = [vsbp.tile([P, NTC, 256], fp16, tag=f"v{q}",
                                      name=f"vsb{q}") for q in range(4)]
                    ot = [otp.tile([P, T], fp16, tag=f"ot{h}",
                                   name=f"ot{h}") for h in range(HG)]

                    # ---------- Phase A-V: V projection, SBUF-resident ------
                    with tc.tile_pool(name="awv", bufs=2) as awvp, \
                         tc.tile_pool(name="avpsum", bufs=4,
                                      space="PSUM") as avpsum:
                        wvq0 = awvp.tile([P, NCS, 256], fp16, tag="awv")
                        nc.sync.dma_start(wvq0, wv_d[:, :, 0:256])
                        xt = []
                        for cs in range(NCS):
                            t_ = xtp.tile([P, T], fp16, tag=f"xt{cs}")
                            nc.sync.dma_start(t_, xt_d[cs])
                            xt.append(t_)
                        for q in range(4):
                            if q == 0:
                                wvq = wvq0
                            else:
                                wvq = awvp.tile([P, NCS, 256], fp16, tag="awv")
                                nc.sync.dma_start(
                                    wvq, wv_d[:, :, q * 256:(q + 1) * 256])
                            for tch in range(NTC):
                                ps = avpsum.tile([P, 256], f32, tag="apv")
                                for cs in range(NCS):
                                    nc.tensor.matmul(
                                        ps, xt[cs][:, tch * P:(tch + 1) * P],
                                        wvq[:, cs],
                                        start=(cs == 0), stop=(cs == NCS - 1))
                                nc.scalar.copy(out=v_sb[q][:, tch], in_=ps)

                    # ---------- per head: QK projection then attention ------
                    with tc.tile_pool(name="aw", bufs=2) as awp, \
                         tc.tile_pool(name="qk", bufs=4) as qkp, \
                         tc.tile_pool(name="bpt", bufs=4) as bptp, \
                         tc.tile_pool(name="bacc", bufs=2) as baccp, \
                         tc.tile_pool(name="bmisc", bufs=4) as bmisc, \
                         tc.tile_pool(name="qkpsum", bufs=2,
                                      space="PSUM") as qkpsum, \
                         tc.tile_pool(name="bpair", bufs=2,
                                      space="PSUM") as bpairp, \
                         tc.tile_pool(name="bpo", bufs=2,
                                      space="PSUM") as bpop:
                        for h in range(HG):
                            # QK projection for head h
                            qt_t = qkp.tile([P, T], fp16, tag="qt")
                            kt_t = qkp.tile([P, T], fp16, tag="kt")
                            for w_d, dst in ((wq_d, qt_t), (wk_d, kt_t)):
                                wt = awp.tile([P, NCS, D], fp16, tag="aw")
                                nc.sync.dma_start(wt, w_d[h])
                                for tb in range(NTB):
                                    ps = qkpsum.tile([P, 512], f32, tag="aps")
                                    for cs in range(NCS):
                                        nc.tensor.matmul(
                                            ps, wt[:, cs],
                                            xt[cs][:, tb * 512:(tb + 1) * 512],
                                            start=(cs == 0),
                                            stop=(cs == NCS - 1))
                                    nc.scalar.copy(
                                        out=dst[:, tb * 512:(tb + 1) * 512],
                                        in_=ps)

                            if not do_b:
                                continue
                            # attention for head h
                            vq = v_sb[h // 2]
                            hd = (h % 2) * D

                            for jb in range(NTB):   # q-blocks of 512
                                # units: pairs of k-chunks sharing one psum
                                # tile + one exp.  (ks, q0, pack_off, width)
                                units = []
                                for i in range(2 * jb):    # off-diagonal
                                    units.append(
                                        ((2 * i, 0, 0, 512),
                                         (2 * i + 1, 0, 512, 512), None))
                                kd = 4 * jb
                                units.append(
                                    ((kd, 0, 0, 512),
                                     (kd + 1, 128, 512, 384), maska_t))
                                units.append(
                                    ((kd + 2, 256, 0, 256),
                                     (kd + 3, 384, 256, 128), maskb_t))

                                po = bpop.tile([P, 512], f32, tag="po")
                                pt_acc = baccp.tile([P, 512], fp16,
                                                    tag="pacc")
                                n_u = len(units)
                                pts = {}

                                def s_step(ui):
                                    sa, sb_, mask = units[ui]
                                    tw = sb_[2] + sb_[3]
                                    pst = bpairp.tile([P, 1024], f32,
                                                      tag="pst")
                                    for (ks, q0, off, w) in (sa, sb_):
                                        nc.tensor.matmul(
                                            pst[:, off:off + w],
                                            kt_t[:, ks * P:(ks + 1) * P],
                                            qt_t[:, jb * 512 + q0:
                                                 jb * 512 + q0 + w],
                                            start=True, stop=True)
                                    pt = bptp.tile([P, 1024], fp16, tag="pt")
                                    nc.scalar.activation(
                                        pt[:, 0:tw], pst[:, 0:tw], EXP,
                                        scale=SCALE)
                                    if mask is not None:
                                        nc.vector.tensor_tensor(
                                            pt[:, 0:tw], pt[:, 0:tw],
                                            mask[:, 0:tw], MULT)
                                    pts[ui] = pt

                                s_step(0)
                                for ui in range(n_u):
                                    if ui + 1 < n_u:
                                        s_step(ui + 1)
                                    sa, sb_, mask = units[ui]
                                    pt = pts.pop(ui)
                                    for si, (ks, q0, off, w) in enumerate(
                                            (sa, sb_)):
                                        nc.tensor.matmul(
                                            po[:, q0:q0 + w],
                                            vq[:, ks, hd:hd + D],
                                            pt[:, off:off + w],
                                            start=(ui == 0 and si == 0),
                                            stop=(ui == n_u - 1 and si == 1))
                                        if ui == 0 and si == 0:
                                            nc.vector.tensor_copy(
                                                out=pt_acc, in_=pt[:, 0:512])
                                        else:
                                            nc.vector.tensor_tensor(
                                                pt_acc[:, q0:q0 + w],
                                                pt_acc[:, q0:q0 + w],
                                                pt[:, off:off + w], ADD)

                                # normalization, all off-PE
                                rs = bmisc.tile([P, 512], fp16, tag="rs")
                                nc.gpsimd.partition_all_reduce(
                                    rs, pt_acc, 128, reduce_op=RADD)
                                rcp = bmisc.tile([P, 512], fp16, tag="rcp")
                                with nc.allow_low_precision("fp16 recip"):
                                    nc.vector.reciprocal(rcp, rs)
                                nc.vector.tensor_tensor(
                                    ot[h][:, jb * 512:(jb + 1) * 512], po,
                                    rcp, MULT)

                    # ------------- Phase C: output projection --------------
                    if not do_c:
                        with tc.tile_pool(name="dummy", bufs=1) as dp:
                            z = dp.tile([P, 512], f32, tag="z")
                            nc.vector.memset(z, 0.0)
                            for tch in range(NTC):
                                for ob in range(4):
                                    nc.sync.dma_start(
                                        out_d[tch * P:(tch + 1) * P,
                                              ob * 512:(ob + 1) * 512], z)
                    else:
                        wo_r = wo_d.rearrange("(h p) o -> p h o", p=P)
                        with tc.tile_pool(name="cw", bufs=2) as cwp, \
                             tc.tile_pool(name="cstage", bufs=4) as cstage, \
                             tc.tile_pool(name="cpsum", bufs=4,
                                          space="PSUM") as cps:
                            for ob in range(4):
                                wo_t = cwp.tile([P, HG, 512], fp16, tag="cw")
                                nc.sync.dma_start(
                                    wo_t, wo_r[:, :, ob * 512:(ob + 1) * 512])
                                for tch in range(NTC):
                                    ps = cps.tile([P, 512], f32, tag="cps")
                                    for h in range(HG):
                                        nc.tensor.matmul(
                                            ps,
                                            ot[h][:, tch * P:(tch + 1) * P],
                                            wo_t[:, h],
                                            start=(h == 0),
                                            stop=(h == HG - 1))
                                    st = cstage.tile([P, 512], f32, tag="cst")
                                    nc.scalar.copy(out=st, in_=ps)
                                    nc.sync.dma_start(
                                        out_d[tch * P:(tch + 1) * P,
                                              ob * 512:(ob + 1) * 512], st)

    nc.finalize()
    return nc


VMODE = "v3"


def _get_nc():
    global _CACHED
    if _CACHED is None:
        _CACHED = _build()
    return _CACHED


def _prep_inputs(x, W_qkv, W_out, vmode=None):
    """Host-side shard + layout prep. Returns per-core input maps."""
    f16 = np.float16
    f32 = np.float32
    x = np.asarray(x, dtype=f32)
    W_qkv = np.asarray(W_qkv, dtype=f32)
    W_out = np.asarray(W_out, dtype=f32)

    # packed causal masks for the two diagonal pair-units of a 512 q-block:
    # pairA = m0 cols[0:512] ++ m1 cols[128:512]; pairB = m2 cols[256:512]
    # ++ m3 cols[384:512]  (mask_m[k, q] = q >= m*128 + k)
    k_idx = np.arange(P)
    q_idx = np.arange(512)
    m = [(q_idx[None, :] >= (mm * P + k_idx)[:, None]).astype(f16)
         for mm in range(4)]
    maska = np.concatenate([m[0], m[1][:, 128:]], axis=1)          # [P, 896]
    maskb = np.concatenate([m[2][:, 256:], m[3][:, 384:]], axis=1)  # [P, 384]

    per_g = {}
    for g in range(2):
        sl = slice(g * HG * D, (g + 1) * HG * D)
        wq = W_qkv[0 * C:1 * C][sl]        # [1024, 2048]
        wk = W_qkv[1 * C:2 * C][sl]
        wv = W_qkv[2 * C:3 * C][sl]
        # [h, p, cs, m]: element = w[h*128+m, cs*128+p]
        wq_a = np.ascontiguousarray(
            wq.reshape(HG, D, NCS, P).transpose(0, 3, 2, 1)).astype(f16)
        wk_a = np.ascontiguousarray(
            wk.reshape(HG, D, NCS, P).transpose(0, 3, 2, 1)).astype(f16)
        # [p, cs, hm]: element = wv[hm, cs*128+p]
        wv_a = np.ascontiguousarray(
            wv.reshape(HG * D, NCS, P).transpose(2, 1, 0)).astype(f16)
        wo_a = np.ascontiguousarray(W_out[:, sl].T).astype(f16)  # [1024, 2048]
        per_g[g] = (wq_a, wk_a, wv_a, wo_a)

    in_maps = []
    for core in range(8):
        b, g = divmod(core, 2)
        xt = np.ascontiguousarray(x[b].T).reshape(NCS, P, T).astype(f16)
        wq_a, wk_a, wv_a, wo_a = per_g[g]
        im = {
            "xt": xt, "wq": wq_a, "wk": wk_a, "wv": wv_a, "wo": wo_a,
            "maska": maska, "maskb": maskb,
        }
        in_maps.append(im)
    return in_maps


def kernel(x, W_qkv, W_out, *, trace=False, trace_cores=None):
    from concourse.bass_utils import run_bass_kernel_spmd

    nc = _get_nc()
    in_maps = _prep_inputs(x, W_qkv, W_out)
    r = run_bass_kernel_spmd(
        nc, in_maps, core_ids=list(range(8)),
        trace=trace, trace_cores=trace_cores)

    out = np.empty((B, T, C), dtype=np.float32)
    for b in range(B):
        out[b] = r.results[2 * b]["out"] + r.results[2 * b + 1]["out"]
    if trace:
        kernel.last_results = r
    return out
